# revision 1
# baseline (speedup 1.0000x reference)
"""HT2IM scatter kernel for Trainium2 (8 NeuronCores, SPMD).

Math: out[ch, p] += ht[ch, q] * w  for each vote (q=ht_index[v], p=im_index[v]),
      ch ranges over B*C=256 channels, q < 10980 (HT pixels), p < 16384 (IM pixels).

Device formulation: out[ch, p] = sum_q ht_T[q, ch] * S[q, p] with the sparse
vote-aggregate matrix S[q, p] = sum_v w_v [q_v=q][p_v=p] built on-chip per call.

Sharding: output pixels split 8 ways (2048 columns per core); every core keeps
the full ht_T (bf16, SBUF) and receives only the votes landing in its slice.

Per core the q axis (padded to 11008) is processed as 43 pairs of 128-row
stripes. For each pair j a [128, 4096] bf16 SBUF tile holds S rows
q in [256j, 256j+256) x 2048 p-columns (stripe s01 at column offset 2048*s01).
The tile is zeroed (DVE), filled with a single SBUF-dst dma_scatter_add
(GPSIMD SWDGE + SDMA CCE-add; 64-byte rows carrying up to 32 adjacent
weights), then consumed by 16 bf16 matmuls (PE) accumulating
psum[ch_half, 2048 p] over all 86 stripes.  Everything is double-buffered so
PE, DVE, GPSIMD and the DMA rings run concurrently.

Host side only bins/packs the integer indices (and resolves duplicate (q,p)
pairs by summing their weights - required because the scatter's CCE add is
not atomic across DMA engines).
"""

import numpy as np
import ml_dtypes

import concourse.bass as bass
from concourse import bacc
from concourse import mybir
from concourse import bass_utils

BF16 = ml_dtypes.bfloat16

B, C = 4, 64
CH = B * C                  # 256 channels
HT_H, HT_W = 183, 60
Q = HT_H * HT_W             # 10980
QP = 11008                  # padded to 86*128
NSTRIPE = 86
NPAIR = 43                  # stripe pairs (256 q rows each)
IM_H, IM_W = 128, 128
P = IM_H * IM_W             # 16384
NCORES = 8
PSL = P // NCORES           # 2048 pixel columns per core
ELEM = 32                   # bf16 elements per scatter row (64 B)
CAP = 4096                  # scatter row capacity per (core, pair) call

_cache = {}


def _build_nc():
    if "nc" in _cache:
        return _cache["nc"]
    f32 = mybir.dt.float32
    bf16 = mybir.dt.bfloat16
    i16 = mybir.dt.int16

    nc = bacc.Bacc(None, target_bir_lowering=False)
    ht_d = nc.dram_tensor("ht", [128, NSTRIPE * CH], bf16, kind="ExternalInput")
    wrows_d = nc.dram_tensor("wrows", [NPAIR, 128, CAP // 128, ELEM], bf16,
                             kind="ExternalInput")
    idxs_d = nc.dram_tensor("idxs", [NPAIR, 128, CAP // 16], i16,
                            kind="ExternalInput")
    i32 = mybir.dt.int32
    cnts_d = nc.dram_tensor("cnts", [1, 64], i32, kind="ExternalInput")
    out_d = nc.dram_tensor("out", [2, 128, PSL], f32, kind="ExternalOutput")

    from contextlib import ExitStack
    ctx = ExitStack()
    with ctx:
        ht_sb = ctx.enter_context(nc.sbuf_tensor("k_htsb", [128, NSTRIPE * CH], bf16))
        wb = ctx.enter_context(nc.sbuf_tensor("k_wb", [128, 4, CAP // 128, ELEM], bf16))
        ib = ctx.enter_context(nc.sbuf_tensor("k_ib", [128, 4, CAP // 16], i16))
        sbuf_s = ctx.enter_context(nc.sbuf_tensor("k_sbs", [128, 4, 2 * PSL], bf16))
        dummy = ctx.enter_context(nc.sbuf_tensor("k_dummy", [128, 4, 2 * PSL], bf16))
        cnt_sb = ctx.enter_context(nc.sbuf_tensor("k_cnt", [1, 64], i32))
        st0 = ctx.enter_context(nc.sbuf_tensor("k_st0", [128, PSL], f32))
        st1 = ctx.enter_context(nc.sbuf_tensor("k_st1", [128, PSL], f32))
        ps0 = ctx.enter_context(nc.psum_tensor("k_ps0", [128, PSL], f32))
        ps1 = ctx.enter_context(nc.psum_tensor("k_ps1", [128, PSL], f32))

        s_ht = ctx.enter_context(nc.semaphore("s_ht"))
        s_ht2 = ctx.enter_context(nc.semaphore("s_ht2"))
        s_cnt = ctx.enter_context(nc.semaphore("s_cnt"))
        s_w = [ctx.enter_context(nc.semaphore(f"s_w{i}")) for i in range(4)]
        s_ms = ctx.enter_context(nc.semaphore("s_ms"))
        s_sc = [ctx.enter_context(nc.semaphore(f"s_sc{i}")) for i in range(4)]
        s_mm = ctx.enter_context(nc.semaphore("s_mm"))
        s_cp = ctx.enter_context(nc.semaphore("s_cp"))
        s_cp2 = ctx.enter_context(nc.semaphore("s_cp2"))
        s_out = ctx.enter_context(nc.semaphore("s_out"))

        with nc.Block() as block:

            @block.sync
            def _(sync):
                sync.dma_start(cnt_sb[:], cnts_d[:]).then_inc(s_cnt, 16)
                sync.dma_start(ht_sb[:, :8 * 2 * CH], ht_d[:, :8 * 2 * CH]).then_inc(s_ht, 16)
                sync.dma_start(ht_sb[:, 8 * 2 * CH:], ht_d[:, 8 * 2 * CH:]).then_inc(s_ht2, 16)
                for j in range(NPAIR):
                    if j >= 4:
                        # wb/ib buffer reuse: scatter j-3 must have drained
                        sync.wait_ge(s_sc[j % 4], 16 * (j // 4))
                    sync.dma_start(wb[:, j % 4], wrows_d[j]).then_inc(s_w[j % 4], 16)
                    sync.dma_start(ib[:, j % 4], idxs_d[j]).then_inc(s_w[j % 4], 16)
                sync.wait_ge(s_cp, 1)
                sync.dma_start(out_d[0], st0[:]).then_inc(s_out, 16)
                sync.wait_ge(s_cp2, 1)
                sync.dma_start(out_d[1], st1[:]).then_inc(s_out, 16)
                sync.wait_ge(s_out, 32)

            @block.vector
            def _(vector):
                for j in range(NPAIR):
                    if j >= 4:
                        # stripe buffer reuse: matmuls of pair j-3 done
                        vector.wait_ge(s_mm, j - 3)
                    vector.memset(sbuf_s[:, j % 4], 0.0).then_inc(s_ms, 1)
                vector.wait_ge(s_mm, NPAIR)
                vector.tensor_copy(st0[:], ps0[:]).then_inc(s_cp, 1)

            @block.scalar
            def _(scalar):
                scalar.wait_ge(s_mm, NPAIR)
                scalar.copy(st1[:], ps1[:]).then_inc(s_cp2, 1)

            @block.gpsimd
            def _(gpsimd):
                from concourse import library_config
                gpsimd.load_library(library_config.mlp)
                r_n = gpsimd.alloc_register("r_cnt")
                gpsimd.wait_ge(s_cnt, 16)
                for j in range(NPAIR):
                    gpsimd.wait_ge(s_w[j % 4], 32 * (j // 4 + 1))
                    gpsimd.wait_ge(s_ms, j + 1)
                    gpsimd.reg_load(r_n, cnt_sb[:1, j:j + 1])
                    gpsimd.dma_scatter_add(
                        sbuf_s[:, j % 4],
                        wb[:, j % 4],
                        ib[:, j % 4],
                        num_idxs=CAP,
                        num_idxs_reg=r_n,
                        elem_size=ELEM,
                        sbuf_tokens_per_rank=128,
                        parity_reg=0,
                        out_ap_other=dummy[:, j % 4],
                    ).then_inc(s_sc[j % 4], 16)

            @block.tensor
            def _(tensor):
                tensor.wait_ge(s_ht, 16)
                for j in range(NPAIR):
                    if j == 8:
                        tensor.wait_ge(s_ht2, 16)
                    tensor.wait_ge(s_sc[j % 4], 16 * (j // 4 + 1))
                    for s01 in range(2):
                        a = 2 * j + s01
                        for h in range(2):
                            lhsT = ht_sb[:, a * CH + h * 128:a * CH + h * 128 + 128]
                            ps = ps0 if h == 0 else ps1
                            for n in range(4):
                                mm = tensor.matmul(
                                    ps[:, n * 512:(n + 1) * 512],
                                    lhsT,
                                    sbuf_s[:, j % 4,
                                           s01 * PSL + n * 512:
                                           s01 * PSL + (n + 1) * 512],
                                    start=(a == 0),
                                    stop=(a == NSTRIPE - 1),
                                )
                    mm.then_inc(s_mm, 1)

    nc.compile()
    _cache["nc"] = nc
    return nc


def _preprocess(input_ht, ht_index, im_index, weight):
    """Bin votes by (core, stripe-pair), dedup (q,p) pairs, pack scatter rows."""
    q = ht_index.astype(np.int64)
    p = im_index.astype(np.int64)
    w = weight.astype(np.float32)

    core = p >> 11
    p_loc = p & (PSL - 1)
    j = q >> 8                      # stripe pair
    b = q & 127                     # partition row
    s01 = (q >> 7) & 1
    col = (s01 << 11) | p_loc       # 0..4095 within the pair tile
    g = col >> 5                    # 64-byte slot
    idx16 = (g << 8) | b            # scatter idx (parity bit 7 = 0)

    callid = core * NPAIR + j
    rowkey = (callid << 15) | idx16
    uniq, inv = np.unique(rowkey, return_inverse=True)
    R = uniq.shape[0]
    rows = np.zeros((R, ELEM), np.float32)
    np.add.at(rows, (inv, col & (ELEM - 1)), w)

    u_call = (uniq >> 15).astype(np.int64)
    u_idx16 = (uniq & 32767).astype(np.int16)
    counts = np.bincount(u_call, minlength=NCORES * NPAIR)
    if counts.max() > CAP:
        raise RuntimeError(f"scatter capacity exceeded: {counts.max()} > {CAP}")
    starts = np.zeros(NCORES * NPAIR, np.int64)
    starts[1:] = np.cumsum(counts)[:-1]
    pos = np.arange(R) - starts[u_call]

    wrows = np.zeros((NCORES, NPAIR, 128, CAP // 128, ELEM), BF16)
    u_core = u_call // NPAIR
    u_j = u_call % NPAIR
    wrows[u_core, u_j, pos % 128, pos // 128, :] = rows.astype(BF16)

    idxs_flat = np.full((NCORES, NPAIR, CAP), -1, np.int16)
    idxs_flat[u_core, u_j, pos] = u_idx16
    # vote i's idx lives at partition i%16, column i//16; replicate across the
    # eight 16-partition groups (one copy per Q7 core)
    idxs_wrapped = idxs_flat.reshape(NCORES, NPAIR, CAP // 16, 16) \
                            .transpose(0, 1, 3, 2)
    idxs_dev = np.ascontiguousarray(
        np.tile(idxs_wrapped, (1, 1, 8, 1)))          # [8, 43, 128, 256]

    # ht_T in stripe layout: ht_sb[b, a*256+ch] = ht[ch, a*128+b]
    htq = np.asarray(input_ht, np.float32).reshape(CH, Q)
    htT = np.zeros((QP, CH), np.float32)
    htT[:Q] = htq.T
    ht_dev = np.ascontiguousarray(
        htT.reshape(NSTRIPE, 128, CH).transpose(1, 0, 2)
           .reshape(128, NSTRIPE * CH)).astype(BF16)

    cnts = np.zeros((NCORES, 1, 64), np.int32)
    cnts[:, 0, :NPAIR] = counts.reshape(NCORES, NPAIR)
    return ht_dev, wrows, idxs_dev, cnts


def kernel(input_ht, ht_index, im_index, weight):
    input_ht = np.asarray(input_ht, dtype=np.float32)
    ht_index = np.asarray(ht_index)
    im_index = np.asarray(im_index)
    weight = np.asarray(weight, dtype=np.float32)
    ht_dev, wrows, idxs_dev, cnts = _preprocess(input_ht, ht_index, im_index, weight)
    nc = _build_nc()
    in_maps = [
        {"ht": ht_dev,
         "wrows": np.ascontiguousarray(wrows[k]),
         "idxs": idxs_dev[k],
         "cnts": cnts[k]}
        for k in range(NCORES)
    ]
    res = bass_utils.run_bass_kernel_spmd(nc, in_maps, core_ids=list(range(NCORES)))
    out = np.empty((CH, P), np.float32)
    for k in range(NCORES):
        out[:, k * PSL:(k + 1) * PSL] = res.results[k]["out"].reshape(CH, PSL)
    return out.reshape(B, C, IM_H, IM_W)



# revision 5
# speedup vs baseline: 1.3452x; 1.3452x over previous
"""HT2IM scatter kernel for Trainium2 (8 NeuronCores, SPMD).

Math: out[ch, p] += ht[ch, q] * w  for each vote (q=ht_index[v], p=im_index[v]),
      ch over B*C=256 channels, q < 10980 (HT pixels), p < 16384 (IM pixels).

Device formulation: out[ch, p] = sum_q ht_T[q, ch] * S[q, p] with the sparse
vote-aggregate matrix S[q, p] = sum_v w_v [q_v=q][p_v=p].

Sharding: output pixels split 8 ways (2048 columns per core); every core keeps
the full ht_T (bf16 stationary, SBUF) and a dense fp8-e3m4 copy of its S slice.

S is built DENSE on the host (pure index binning + dtype packing, no float
math beyond summing duplicate-cell weights, same as the reference's
segment-sum semantics) as 86 q-stripes of [128, 2048] e3m4, streamed
HBM->SBUF through a 4-deep buffer ring at full DMA rate (2KB contiguous per
partition per tile).  The PE consumes each stripe with 8 matmuls
(psum[128ch, 512p] += htT[128q, 128ch].T @ S[128q, 512p]) accumulating over
all 86 stripes; moving operand is e3m4 (1 cycle/row), stationary is bf16
(mixed-dtype matmul, verified exact on HW).  S carries 2*w and ht carries
ht/2 (exact exponent shifts) to center w's e3m4 exponent window; accumulation
is fp32 in PSUM.  rel err ~1.3e-2 vs fp32 reference (e3m4 quantization of w).

The kernel is DMA-light (S 21.5MB + ht 5.5MB + out 1MB per core ~= 82us of
DMA) and PE-bound (86*8 matmuls of 512 cols ~= 147us), so the S stream and
the interleaved ht chunks hide completely behind the matmul pipeline.
"""

import numpy as np
import ml_dtypes

import concourse.bass as bass
from concourse import bacc
from concourse import mybir
from concourse import bass_utils

BF16 = ml_dtypes.bfloat16
E3M4 = ml_dtypes.float8_e3m4

B, C = 4, 64
CH = B * C                  # 256 channels
HT_H, HT_W = 183, 60
Q = HT_H * HT_W             # 10980
QP = 11008                  # padded to 86*128
NSTRIPE = 86
IM_H, IM_W = 128, 128
P = IM_H * IM_W             # 16384
NCORES = 8
PSL = P // NCORES           # 2048 pixel columns per core
HT_CHUNK = 8                # ht stripes per DMA chunk
NHTCH = (NSTRIPE + HT_CHUNK - 1) // HT_CHUNK   # 11 chunks

_cache = {}


def _build_nc():
    if "nc" in _cache:
        return _cache["nc"]
    f32 = mybir.dt.float32
    bf16 = mybir.dt.bfloat16
    f8e3 = mybir.dt.float8e3

    nc = bacc.Bacc(None, target_bir_lowering=False)
    ht_d = nc.dram_tensor("ht", [128, NSTRIPE * CH], bf16, kind="ExternalInput")
    s_d = nc.dram_tensor("s", [NSTRIPE, 128, PSL], f8e3, kind="ExternalInput")
    out_d = nc.dram_tensor("out", [2, 128, PSL], bf16, kind="ExternalOutput")

    from contextlib import ExitStack
    ctx = ExitStack()
    with ctx:
        ht_sb = ctx.enter_context(nc.sbuf_tensor("k_htsb", [128, NSTRIPE * CH], bf16))
        s_sb = ctx.enter_context(nc.sbuf_tensor("k_ssb", [128, 4, PSL], f8e3))
        st0 = ctx.enter_context(nc.sbuf_tensor("k_st0", [128, PSL], bf16))
        st1 = ctx.enter_context(nc.sbuf_tensor("k_st1", [128, PSL], bf16))
        ps0 = ctx.enter_context(nc.psum_tensor("k_ps0", [128, PSL], f32))
        ps1 = ctx.enter_context(nc.psum_tensor("k_ps1", [128, PSL], f32))

        s_ht = [ctx.enter_context(nc.semaphore(f"s_ht{i}")) for i in range(2)]
        s_s = [ctx.enter_context(nc.semaphore(f"s_s{i}")) for i in range(4)]
        s_mm = ctx.enter_context(nc.semaphore("s_mm"))
        s_cp = ctx.enter_context(nc.semaphore("s_cp"))
        s_cp2 = ctx.enter_context(nc.semaphore("s_cp2"))
        s_out = ctx.enter_context(nc.semaphore("s_out"))

        with nc.Block() as block:

            @block.sync
            def _(sync):
                # interleave ht chunks with the S-tile stream so the first
                # matmul starts after ~2 tiles and ht never blocks the ring
                for t in range(NSTRIPE):
                    g = t // HT_CHUNK
                    if t % HT_CHUNK == 0:
                        lo = g * HT_CHUNK * CH
                        hi = min(NSTRIPE * CH, (g + 1) * HT_CHUNK * CH)
                        sync.dma_start(ht_sb[:, lo:hi], ht_d[:, lo:hi]) \
                            .then_inc(s_ht[g % 2], 16)
                    if t >= 4:
                        # buffer ring reuse: matmuls of stripe t-4 done
                        sync.wait_ge(s_mm, t - 3)
                    sync.dma_start(s_sb[:, t % 4], s_d[t]).then_inc(s_s[t % 4], 16)
                sync.wait_ge(s_cp, 1)
                sync.dma_start(out_d[0], st0[:]).then_inc(s_out, 16)
                sync.wait_ge(s_cp2, 1)
                sync.dma_start(out_d[1], st1[:]).then_inc(s_out, 16)
                sync.wait_ge(s_out, 32)

            @block.tensor
            def _(tensor):
                for t in range(NSTRIPE):
                    if t % HT_CHUNK == 0:
                        g = t // HT_CHUNK
                        tensor.wait_ge(s_ht[g % 2], 16 * (g // 2 + 1))
                    tensor.wait_ge(s_s[t % 4], 16 * (t // 4 + 1))
                    for h in range(2):
                        lhsT = ht_sb[:, t * CH + h * 128:t * CH + h * 128 + 128]
                        ps = ps0 if h == 0 else ps1
                        for n in range(4):
                            mm = tensor.matmul(
                                ps[:, n * 512:(n + 1) * 512],
                                lhsT,
                                s_sb[:, t % 4, n * 512:(n + 1) * 512],
                                start=(t == 0),
                                stop=(t == NSTRIPE - 1),
                            )
                    mm.then_inc(s_mm, 1)

            @block.vector
            def _(vector):
                vector.wait_ge(s_mm, NSTRIPE)
                vector.tensor_copy(st0[:], ps0[:]).then_inc(s_cp, 1)

            @block.scalar
            def _(scalar):
                scalar.wait_ge(s_mm, NSTRIPE)
                scalar.copy(st1[:], ps1[:]).then_inc(s_cp2, 1)

    nc.compile()
    _cache["nc"] = nc
    return nc


def _preprocess(input_ht, ht_index, im_index, weight):
    """Pack ht (bf16 stripe layout, x0.5) and dense per-core S tiles (e3m4, x2)."""
    q = ht_index.astype(np.int64)
    p = im_index.astype(np.int64)
    w = weight.astype(np.float32)

    # ht_T in stripe layout: ht_sb[b, t*256+ch] = 0.5*ht[ch, 128t+b]
    htq = np.asarray(input_ht, np.float32).reshape(CH, Q) * 0.5
    htT = np.zeros((QP, CH), np.float32)
    htT[:Q] = htq.T
    ht_dev = np.ascontiguousarray(
        htT.reshape(NSTRIPE, 128, CH).transpose(1, 0, 2)
           .reshape(128, NSTRIPE * CH)).astype(BF16)

    core = p >> 11
    idx_in_core = q * PSL + (p & (PSL - 1))
    s_dev = np.empty((NCORES, NSTRIPE, 128, PSL), E3M4)
    for k in range(NCORES):
        m = core == k
        dense = np.bincount(idx_in_core[m], weights=w[m],
                            minlength=QP * PSL).astype(np.float32)
        dense *= 2.0
        s_dev[k] = dense.astype(E3M4).reshape(NSTRIPE, 128, PSL)
    return ht_dev, s_dev


def kernel(input_ht, ht_index, im_index, weight):
    input_ht = np.asarray(input_ht, dtype=np.float32)
    ht_index = np.asarray(ht_index)
    im_index = np.asarray(im_index)
    weight = np.asarray(weight, dtype=np.float32)
    ht_dev, s_dev = _preprocess(input_ht, ht_index, im_index, weight)
    nc = _build_nc()
    in_maps = [{"ht": ht_dev, "s": s_dev[k]} for k in range(NCORES)]
    res = bass_utils.run_bass_kernel_spmd(nc, in_maps, core_ids=list(range(NCORES)))
    out = np.empty((CH, P), np.float32)
    for k in range(NCORES):
        out[:, k * PSL:(k + 1) * PSL] = \
            res.results[k]["out"].reshape(CH, PSL).astype(np.float32)
    return out.reshape(B, C, IM_H, IM_W)
